# revision 23
# baseline (speedup 1.0000x reference)
"""Trainium2 Bass kernel for nn_Attention (B=2, S=2048, D=1024, H=16).

Sharding: 8 cores = 2 batches x 4 head-groups (4 heads per core).
Each core computes QKV projection for its batch restricted to its 4 heads,
full (non-causal) attention for those heads, and a partial output
projection over its 256 channels. The host sums the 4 partial outputs per
batch.

Design notes (v3, HW ~280us vs 324us baseline; rel err ~5.4e-3):
 - x transposed + bf16-cast on host; wqk/wv/qT/kTe/kTo/v/ering bf16,
   attn/wout f32r, y out bf16 (host upcasts and sums partials in f32).
 - W_q/W_k columns are de-interleaved per head on host ([ev|od] instead
   of interleaved rope pairs) so the DVE rope ops see packed 32-wide
   slices: all-bf16 packed operands hit the DVE 2x_1p mode (~2x faster).
   q/k live in this permuted basis everywhere; logits are invariant.
 - kT is stored as kTe/kTo with zero-padded dead halves so every logits
   matmul contracts over K=128: measured on HW, K=64 matmuls run at half
   throughput (482 vs 251 ns for 512 moving cols).
 - PSUM = two pools of rotating tags (no other banks exist): psl holds
   L0/L1/L2 (stage A qk-pair proj + q transposes; stage B triple-buffered
   per-window logits tiles + divide/out-proj scratch on the idle slot —
   scratch tag (W+2)%3 must avoid kt = 2 mod 3 or it collides with the
   next unit's prologue slot), pso holds O (stage A v proj + k
   transposes; stage B AV accumulator). Triple buffering gives every
   producer->consumer sem hop >= 1 full window of slack.
 - stage A per 512-token group, PE order qk x4 -> Tq(g-1) -> v x4 ->
   Tk(g-1): qk pairs drain (ACT) to bf16 qk_sb, DVE ropes into qk_r;
   transposes are deferred one group so PE never waits on rope; Tk after
   v keeps the O-tag rotation off v's critical path. All PSUM drains on
   ACT (a DVE drain queues behind rope and stalls PE).
 - stage B per (query-chunk, head-pair) unit: per key tile kt one
   1024-wide bf16 exp covers both heads; logits(kt+1) issued before
   AV(kt); AV lags one window (flush_av) so the O-drain -> next-unit-AV
   rotation has slack. ACT runs exp back-to-back: stage B sits at the
   ACT roofline (~1.3us/window on HW).
 - divide: O drained to SBUF at unit end; exact DVE reciprocal (~6.9us
   on HW for 1024 - the approx custom op miscompiles here) runs in DVE
   slack, divide2 (K=1 broadcast matmuls + partition-shifted muls) rides
   at kt==10 of the next unit. Out-proj units ride tag-safe windows
   after attn(qc-1) is fully divided; only qc=3 remains in the tail.
   y is staged per query chunk and DMA'd as 4 big transfers (each HWDGE
   dispatch costs ~1.25us of SEQ time - 32 small DMAs would serialize).
 - DMA: only SP/ACT have HWDGE queues, so: wqk 3 DMAs on ACT, xt(g0) 3
   on SP (finer splits let the first chain start ~2us in), cos|sin
   merged into one [S,64] bf16 tensor on gpsimd SWDGE, wout last.
"""

import numpy as np

S = 2048
D = 1024
HD = 64
H_LOC = 4  # heads per core
N_CORES = 8
TT = 16  # token tiles of 128
G = 4    # token groups of 512
QC = 4   # query chunks of 512
KT = 16  # key tiles of 128

_CACHED = {}


def build_nc(repeats: int = 1, with_bias: bool = False):
    import concourse.bass as bass_mod
    import concourse.mybir as mybir
    from concourse import bacc
    from concourse.tile import TileContext
    f32 = mybir.dt.float32
    f32r = mybir.dt.float32r
    bf16 = mybir.dt.bfloat16
    Exp = mybir.ActivationFunctionType.Exp

    nc = bacc.Bacc("TRN2", target_bir_lowering=False, debug=False,
                   num_devices=N_CORES)

    xt_d = nc.dram_tensor("xt", [D, S], bf16, kind="ExternalInput")
    cs_d = nc.dram_tensor("cs", [S, 64], bf16, kind="ExternalInput")
    wqk_d = nc.dram_tensor("wqk", [D, 512], bf16, kind="ExternalInput")
    wv_d = nc.dram_tensor("wv", [D, 256], bf16, kind="ExternalInput")
    wout_d = nc.dram_tensor("wout", [256, D], f32r, kind="ExternalInput")
    ones_d = nc.dram_tensor("ones", [1, 128], f32r, kind="ExternalInput")
    onescol_d = nc.dram_tensor("onescol", [128, 64], f32r,
                               kind="ExternalInput")
    ident_d = nc.dram_tensor("ident", [128, 128], bf16, kind="ExternalInput")
    if with_bias:
        bqk_d = nc.dram_tensor("bqk", [1, 512], bf16, kind="ExternalInput")
        bv_d = nc.dram_tensor("bv", [1, 256], bf16, kind="ExternalInput")
        bout_d = nc.dram_tensor("bout", [1, D], f32r, kind="ExternalInput")
    y_d = nc.dram_tensor("y", [S, D], bf16, kind="ExternalOutput")

    with TileContext(nc) as tc:
        with (
            tc.tile_pool(name="const", bufs=1) as cpool,
            tc.tile_pool(name="xin", bufs=2) as xpool,
            tc.tile_pool(name="qkr", bufs=2) as qkrpool,
            tc.tile_pool(name="rtmp", bufs=2) as rtpool,
            tc.tile_pool(name="big", bufs=1) as bigpool,
            tc.tile_pool(name="et", bufs=6) as etpool,
            tc.tile_pool(name="yt", bufs=2) as ypool,
            tc.tile_pool(name="ysb", bufs=2) as ysbpool,
            tc.tile_pool(name="tl", bufs=1) as tailpool,
            tc.tile_pool(name="sml", bufs=1) as spool,
            tc.tile_pool(name="psl", bufs=1, space="PSUM") as psl,
            tc.tile_pool(name="pso", bufs=1, space="PSUM") as pso,
        ):
            # ---- constants / weights ----
            wqk_sb = cpool.tile([128, 8, 512], bf16)
            wv_sb = cpool.tile([128, 8, 256], bf16)
            wout_sb = cpool.tile([128, 2, D], f32r)
            cs_sb = cpool.tile([128, TT, 64], bf16)
            ones_sb = cpool.tile([1, 128], f32r)
            onescol_sb = cpool.tile([128, 64], f32r)
            ident = cpool.tile([128, 128], bf16)
            if with_bias:
                bqk_sb = cpool.tile([1, 512], bf16)
                bv_sb = cpool.tile([1, 256], bf16)
                bout_sb = cpool.tile([1, D], f32r)
                ones_bf = cpool.tile([1, 128], bf16)

            # wqk on the ACT queue (2 dispatches, done before the first
            # drain is needed); everything else on gpsimd SWDGE ordered by
            # first use (cs for rope ~4us, wv for v-proj ~7us, ident for
            # transposes ~10us, wout not until stage B).
            wqk_r = wqk_d.ap().rearrange("(i p) c -> p i c", p=128)
            nc.scalar.dma_start(wqk_sb[:, 0:2, :], wqk_r[:, 0:2, :])
            nc.scalar.dma_start(wqk_sb[:, 2:4, :], wqk_r[:, 2:4, :])
            nc.scalar.dma_start(wqk_sb[:, 4:8, :], wqk_r[:, 4:8, :])
            nc.gpsimd.dma_start(cs_sb[:], cs_d.ap().rearrange(
                "(t p) c -> p t c", p=128))
            nc.gpsimd.dma_start(wv_sb[:], wv_d.ap().rearrange(
                "(i p) c -> p i c", p=128))
            nc.gpsimd.dma_start(ident[:], ident_d[:])
            nc.gpsimd.dma_start(onescol_sb[:], onescol_d[:])
            nc.gpsimd.dma_start(ones_sb[:], ones_d[:])
            nc.gpsimd.dma_start(wout_sb[:], wout_d.ap().rearrange(
                "(i p) c -> p i c", p=128))
            if with_bias:
                nc.gpsimd.dma_start(bqk_sb[:], bqk_d[:])
                nc.gpsimd.dma_start(bv_sb[:], bv_d[:])
                nc.gpsimd.dma_start(bout_sb[:], bout_d[:])
                nc.vector.tensor_copy(ones_bf[:], ones_sb[:])

            def bcast8(ap):
                # [p, t, j] -> [p, t, (bcast 8), j]
                return bass_mod.AP(ap.tensor, ap.offset,
                                   [ap.ap[0], ap.ap[1], [0, 8], ap.ap[2]])

            def body(_iv=None):
                qT = bigpool.tile([128, 2, S], bf16, tag="qT")
                # kT split per head parity with zero-padded dead halves so
                # logits matmuls contract over K=128 (K=64 runs at half
                # throughput on HW): kTe rows 0:64 = even head, rows 64:128
                # zero; kTo rows 0:64 zero, rows 64:128 = odd head. The
                # moving operand is then the full 128-row packed qT — the
                # other head's rows hit the zero stationary rows.
                kTe = bigpool.tile([128, 2, S], bf16, tag="kTe")
                kTo = bigpool.tile([128, 2, S], bf16, tag="kTo")
                attn = bigpool.tile([128, 2, S], f32r, tag="attn")
                v_sb = bigpool.tile([128, TT, H_LOC, 65], bf16, tag="v")
                nc.vector.memset(kTe[64:128, :, :], 0.0)
                nc.vector.memset(kTo[0:64, :, :], 0.0)
                nc.vector.tensor_copy(
                    v_sb[:, :, :, 64:65],
                    onescol_sb[:].rearrange("p (t h o) -> p t h o",
                                            h=H_LOC, o=1))

                # PSUM: 8 banks as two pools of rotating tags. psl holds
                # three 2-bank slots L0/L1/L2 (stage A: qk pair proj + q
                # transposes; stage B: triple-buffered logits + ride/bcast
                # scratch on the rotating idle slot). pso holds one 2-bank
                # slot O (stage A: v proj + k transposes; stage B: AV
                # accumulator). HW semaphore latency is ~0.5us per hop, so
                # every producer-consumer pair needs >= 1 window of slack —
                # triple buffering gives logits->exp 2 windows.

                # ================= stage A: projections =================
                def rope_pair(pr, qk_sb, qk_r, g):
                    # rope for token tiles (2*pr, 2*pr+1); all-bf16 packed
                    # operands (host de-interleaved W cols) -> DVE 2x mode
                    tt = g * 4 + 2 * pr
                    ts = slice(2 * pr, 2 * pr + 2)
                    cosp = bcast8(cs_sb[:, tt:tt + 2, 0:32])
                    sinp = bcast8(cs_sb[:, tt:tt + 2, 32:64])
                    srcr = qk_sb[:, ts, :].rearrange(
                        "p t (h half j) -> p half t h j", half=2, j=32)
                    dstr = qk_r[:, ts, :].rearrange(
                        "p t (h half j) -> p half t h j", half=2, j=32)
                    ev, od = srcr[:, 0], srcr[:, 1]
                    t1 = rtpool.tile([128, 2, 8, 32], bf16, tag="t1")
                    t2 = rtpool.tile([128, 2, 8, 32], bf16, tag="t2")
                    nc.vector.tensor_mul(t1[:], od, sinp)
                    nc.vector.tensor_mul(dstr[:, 0], ev, cosp)
                    nc.vector.tensor_sub(dstr[:, 0], dstr[:, 0], t1[:])
                    nc.vector.tensor_mul(t2[:], ev, sinp)
                    nc.vector.tensor_mul(dstr[:, 1], od, cosp)
                    nc.vector.tensor_add(dstr[:, 1], dstr[:, 1], t2[:])

                def do_transposes_q(gm, qk_r):
                    # deferred to the NEXT group's PE stream so rope(gm) has
                    # a full group of slack — PE never waits on it
                    gb = slice(gm * 512, (gm + 1) * 512)
                    Tq = psl.tile([128, 2, 512], bf16, tag="L2",
                                  name=f"Tq{gm}", padded_shape=[128, 2, 1024])
                    for cc in range(2):
                        ps_t = Tq[:, cc, :]
                        for ti in range(4):
                            nc.tensor.transpose(
                                ps_t[:, ti * 128:(ti + 1) * 128],
                                qk_r[:, ti, cc * 128:(cc + 1) * 128],
                                ident[:])
                    nc.scalar.copy(qT[:, 0:2, gb], Tq[:, :, :])

                def do_transposes_k(gm, qk_r):
                    # after the current group's v projections, so the tag-O
                    # rotation (Oa(g) -> Tk(gm) -> Oa(g+1)) never stalls v
                    gb = slice(gm * 512, (gm + 1) * 512)
                    Tk = pso.tile([128, 2, 512], bf16, tag="O",
                                  name=f"Tk{gm}", padded_shape=[128, 2, 1024])
                    for cc in range(2):
                        ps_t = Tk[:, cc, :]
                        for ti in range(4):
                            nc.tensor.transpose(
                                ps_t[:, ti * 128:(ti + 1) * 128],
                                qk_r[:, ti, (cc + 2) * 128:(cc + 3) * 128],
                                ident[:])
                    # k: rows 0:64 = even head, 64:128 = odd head
                    nc.scalar.copy(kTe[0:64, 0:2, gb], Tk[0:64, :, :])
                    nc.scalar.copy(kTo[64:128, 0:2, gb], Tk[64:128, :, :])

                prev_qkr = [None]

                for g in range(G):
                    xt_g = xpool.tile([128, 8, 512], bf16, tag="xt")
                    xt_r = xt_d[:, g * 512:(g + 1) * 512].rearrange(
                        "(i p) s -> p i s", p=128)
                    if g == 0:
                        nc.sync.dma_start(xt_g[:, 0:2, :], xt_r[:, 0:2, :])
                        nc.sync.dma_start(xt_g[:, 2:4, :], xt_r[:, 2:4, :])
                        nc.sync.dma_start(xt_g[:, 4:8, :], xt_r[:, 4:8, :])
                    else:
                        nc.sync.dma_start(xt_g[:], xt_r)

                    qkA = psl.tile([128, 2, 512], f32, tag="L0",
                                   name=f"qkA{g}")
                    qkB = psl.tile([128, 2, 512], f32, tag="L1",
                                   name=f"qkB{g}")
                    qk_sb = tailpool.tile([128, 4, 512], bf16, tag="qks",
                                          name=f"qks{g}")
                    qk_r = qkrpool.tile([128, 4, 512], bf16, tag="qkr")

                    # 4 qk projections; pair drains (ACT) + ropes (DVE)
                    # overlap the later chains
                    for ti in range(4):
                        ps_qk = (qkA, qkB)[ti // 2][:, ti % 2, :]
                        for fc in range(8):
                            nc.tensor.matmul(
                                ps_qk, xt_g[:, fc, ti * 128:(ti + 1) * 128],
                                wqk_sb[:, fc, :],
                                start=(fc == 0),
                                stop=(not with_bias and fc == 7))
                        if with_bias:
                            nc.tensor.matmul(ps_qk, ones_bf[0:1, 0:128],
                                             bqk_sb[:], start=False, stop=True)
                        if ti % 2 == 1:
                            src = (qkA, qkB)[ti // 2]
                            nc.scalar.copy(
                                qk_sb[:, ti - 1:ti + 1, :], src[:, :, :])
                            rope_pair(ti // 2, qk_sb, qk_r, g)

                    # previous group's q transposes ride here (rope long done)
                    if prev_qkr[0] is not None:
                        do_transposes_q(g - 1, prev_qkr[0])

                    # 4 v projections into O (slot = ti-pair, col = ti%2);
                    # one ACT drain after all four
                    O_a = pso.tile([128, 2, 512], f32, tag="O", name=f"Oa{g}")
                    for ti in range(4):
                        ps_v = O_a[:, ti // 2,
                                   (ti % 2) * 256:(ti % 2) * 256 + 256]
                        for fc in range(8):
                            nc.tensor.matmul(
                                ps_v, xt_g[:, fc, ti * 128:(ti + 1) * 128],
                                wv_sb[:, fc, :],
                                start=(fc == 0),
                                stop=(not with_bias and fc == 7))
                        if with_bias:
                            nc.tensor.matmul(ps_v, ones_bf[0:1, 0:128],
                                             bv_sb[:], start=False, stop=True)
                    nc.scalar.copy(
                        v_sb[:, g * 4:(g + 1) * 4, :, 0:64],
                        O_a[:, :, :].rearrange(
                            "p s (t h d) -> p (s t) h d", t=2, h=H_LOC, d=64))

                    # previous group's k transposes after v (tag-O order)
                    if prev_qkr[0] is not None:
                        do_transposes_k(g - 1, prev_qkr[0])
                    prev_qkr[0] = qk_r

                do_transposes_q(G - 1, prev_qkr[0])
                do_transposes_k(G - 1, prev_qkr[0])

                # ============ stage B: attention (+ C interleaved) ============
                units = [(qc, hp) for qc in range(QC) for hp in range(2)]
                ltiles = {}          # global window index -> logits tile
                scratch_n = [0]      # rotating tag counter for tail psum

                def logits(qc, hp, kt, W):
                    t = psl.tile([128, 2, 512], f32, tag=f"L{W % 3}",
                                 name=f"Lg{W}")
                    ltiles[W] = t
                    for (j, ktp) in ((0, kTe), (1, kTo)):
                        nc.tensor.matmul(
                            t[:, j, :],
                            ktp[:, hp, kt * 128:(kt + 1) * 128],
                            qT[:, hp, qc * 512:(qc + 1) * 512],
                            start=True, stop=True)

                def scratch(W, name):
                    # the idle L slot at window W (exp(W-1) just released it,
                    # logits(W+2) won't claim it until next window)
                    if W is None:
                        scratch_n[0] += 1
                        return psl.tile([128, 2, 512], f32,
                                        tag=f"L{scratch_n[0] % 3}", name=name)
                    return psl.tile([128, 2, 512], f32, tag=f"L{(W + 2) % 3}",
                                    name=name)

                ysb_state = {}   # qc2 -> [tile, n_done]

                def c_unit(qc2, u, W, drain_eng):
                    # full out-proj unit (both halves + drain into the
                    # chunk-local y staging tile; chunk DMA after the 8th)
                    if qc2 not in ysb_state:
                        ysb_state[qc2] = [ysbpool.tile([128, 4, 1024], bf16,
                                                       tag="ysb",
                                                       name=f"ysb{qc2}"), 0]
                    y_sb = ysb_state[qc2][0]
                    ti, ec = divmod(u, 2)
                    tt = qc2 * 4 + ti
                    y_ps = scratch(W, f"ycu{qc2}_{u}")[:, 0, :]
                    for half in range(2):
                        nc.tensor.matmul(
                            y_ps, attn[:, half, tt * 128:(tt + 1) * 128],
                            wout_sb[:, half, ec * 512:(ec + 1) * 512],
                            start=(half == 0),
                            stop=(half == 1 and not with_bias))
                    if with_bias:
                        nc.tensor.matmul(y_ps, ones_sb[0:1, 0:128],
                                         bout_sb[0:1, ec * 512:(ec + 1) * 512],
                                         start=False, stop=True)
                    dst = y_sb[:, ti, ec * 512:(ec + 1) * 512]
                    if drain_eng == "act":
                        nc.scalar.copy(dst, y_ps)
                    else:
                        nc.vector.tensor_copy(dst, y_ps)
                    ysb_state[qc2][1] += 1
                    if ysb_state[qc2][1] == 8:
                        nc.sync.dma_start(
                            y_d[qc2 * 512:(qc2 + 1) * 512, :].rearrange(
                                "(t p) c -> p t c", p=128),
                            y_sb[:])

                def divide1(ui, qc, hp, O):
                    # Drain O to SBUF immediately (frees O for the next
                    # unit's AV accumulation) and take the reciprocal of
                    # the sums row — all DVE, off the PE critical path.
                    o_sb = spool.tile([65, 2, 512], f32r, tag="osb",
                                      name=f"osb{ui}")
                    nc.vector.tensor_copy(o_sb[:, :, :], O[0:65, :, :])
                    rec = spool.tile([1, 1024], f32r, tag="rec",
                                     name=f"rec{ui}")
                    with nc.allow_low_precision(
                            reason="f32r reciprocal feeds f32r multiply"):
                        # exact DVE reciprocal is ~6 cyc/elem on HW (~6.9us
                        # for 1024) but attn is only needed at the tail now,
                        # so divide2 rides late (kt==8) and the DVE queue has
                        # ~10us of slack per unit to absorb it.
                        nc.vector.reciprocal(rec[0:1, :],
                                             o_sb[64:65, :, :].rearrange(
                                                 "p a b -> p (a b)"))
                    return (ui, qc, hp, o_sb, rec)

                def divide2(ui, qc, hp, o_sb, rec, W):
                    # K=1 broadcast matmuls into the two banks of one
                    # scratch slot, then partition-shifted DVE multiplies.
                    bcb = scratch(W, f"bcb{ui}")
                    nc.tensor.matmul(bcb[0:64, 0, :], ones_sb[0:1, 0:64],
                                     rec[0:1, 0:512], start=True, stop=True)
                    nc.tensor.matmul(bcb[0:64, 1, :], ones_sb[0:1, 0:64],
                                     rec[0:1, 512:1024], start=True, stop=True)
                    qs = slice(qc * 512, (qc + 1) * 512)
                    nc.vector.tensor_mul(attn[0:64, hp, qs],
                                         o_sb[0:64, 0, :], bcb[0:64, 0, :])
                    nc.vector.tensor_mul(attn[64:128, hp, qs],
                                         o_sb[0:64, 1, :], bcb[0:64, 1, :])

                logits(0, 0, 0, 0)  # prologue for the first unit
                pdiv = [None]
                # AV matmuls are issued one window late (AV(W-1) inside
                # window W) so the O-drain -> next-unit-AV tag rotation has
                # a full window of slack instead of stalling PE ~1.7us at
                # every unit boundary.
                prev_av = [None]

                def flush_av():
                    O2, ui2, qc2, hp2, kt2, er2 = prev_av[0]
                    nc.tensor.matmul(
                        O2[0:65, 0, :], v_sb[:, kt2, 2 * hp2, :],
                        er2[:, 0, :], start=(kt2 == 0), stop=(kt2 == KT - 1))
                    nc.tensor.matmul(
                        O2[0:65, 1, :], v_sb[:, kt2, 2 * hp2 + 1, :],
                        er2[:, 1, :], start=(kt2 == 0), stop=(kt2 == KT - 1))
                    if kt2 == KT - 1:
                        pdiv[0] = divide1(ui2, qc2, hp2, O2)

                for ui, (qc, hp) in enumerate(units):
                    O = pso.tile([128, 2, 512], f32, tag="O", name=f"O{ui}")
                    if hp == 0 and qc > 0:
                        rides = {12: (qc - 1, 0), 13: (qc - 1, 1)}
                    elif hp == 1 and qc > 0:
                        rides = dict(zip((3, 4, 6, 7, 9, 12),
                                         ((qc - 1, u) for u in range(2, 8))))
                    else:
                        rides = {}
                    for kt in range(KT):
                        W = ui * KT + kt
                        ering = etpool.tile([128, 2, 512], bf16, tag="er")
                        src = ltiles.pop(W)
                        nc.scalar.activation(ering[:], src[:, :, :],
                                             Exp, scale=0.125)
                        if kt < KT - 1:
                            logits(qc, hp, kt + 1, W + 1)
                        elif ui + 1 < len(units):
                            qc2, hp2 = units[ui + 1]
                            with tc.high_priority():
                                logits(qc2, hp2, 0, W + 1)
                        # the previous unit's softmax divide rides at kt==10:
                        # its scratch tag must avoid kt = 2 mod 3 (those
                        # collide with the next unit's prologue logits slot)
                        if kt == 10 and pdiv[0] is not None:
                            divide2(*pdiv[0], W)
                            pdiv[0] = None
                        if kt in rides:
                            qc2, u = rides[kt]
                            c_unit(qc2, u, W, "act" if u % 2 else "dve")
                        if prev_av[0] is not None:
                            flush_av()
                        prev_av[0] = (O, ui, qc, hp, kt, ering)

                flush_av()  # the last window's AV + divide1

                # ---- tail: the last chunk's divide + out-proj ----
                divide2(*pdiv[0], None)
                pdiv[0] = None
                for u in range(8):
                    c_unit(QC - 1, u, None, "act" if u % 2 else "dve")

            if repeats == 1:
                body()
            else:
                with tc.For_i(0, repeats, 1) as _i:
                    body(_i)

    nc.compile()
    return nc


# de-interleave rope pairs within each head's 64 channels: [ev(32)|od(32)]
_DEINT = np.concatenate([np.arange(0, 64, 2), np.arange(1, 64, 2)])


def _prep_in_maps(x, rope_cos, rope_sin, W_qkv, b_qkv, W_out, b_out,
                  with_bias=False):
    import ml_dtypes
    f32 = np.float32
    bfl = ml_dtypes.bfloat16
    W3 = np.asarray(W_qkv, dtype=f32).reshape(D, 16, 3, HD)  # [f, head, qkv, d]
    b3 = np.asarray(b_qkv, dtype=f32).reshape(16, 3, HD)
    cs = np.concatenate([np.asarray(rope_cos, dtype=f32),
                         np.asarray(rope_sin, dtype=f32)], axis=1)
    cs = np.ascontiguousarray(cs).astype(bfl)
    ones = np.ones((1, 128), dtype=f32)
    onescol = np.ones((128, 64), dtype=f32)
    W_out = np.asarray(W_out, dtype=f32)
    b_out = np.asarray(b_out, dtype=f32)
    x = np.asarray(x, dtype=f32)

    in_maps = []
    for c in range(N_CORES):
        b, hg = divmod(c, 4)
        hs = slice(hg * H_LOC, (hg + 1) * H_LOC)
        wq = W3[:, hs, 0, :][:, :, _DEINT].reshape(D, 256)
        wk = W3[:, hs, 1, :][:, :, _DEINT].reshape(D, 256)
        wv = W3[:, hs, 2, :].reshape(D, 256)
        m = {
            "xt": np.ascontiguousarray(x[b].T).astype(bfl),
            "cs": cs,
            "wqk": np.ascontiguousarray(
                np.concatenate([wq, wk], axis=1)).astype(bfl),
            "wv": np.ascontiguousarray(wv).astype(bfl),
            "wout": np.ascontiguousarray(W_out[hg * 256:(hg + 1) * 256, :]),
            "ones": ones, "onescol": onescol,
            "ident": np.eye(128, dtype=f32).astype(bfl),
        }
        if with_bias:
            bq = b3[hs, 0, :][:, _DEINT].reshape(1, 256)
            bk = b3[hs, 1, :][:, _DEINT].reshape(1, 256)
            m["bqk"] = np.ascontiguousarray(
                np.concatenate([bq, bk], axis=1)).astype(bfl)
            m["bv"] = np.ascontiguousarray(
                b3[hs, 2, :].reshape(1, 256)).astype(bfl)
            m["bout"] = (np.ascontiguousarray(b_out.reshape(1, D)) if hg == 0
                         else np.zeros((1, D), dtype=f32))
        in_maps.append(m)
    return in_maps


def kernel(x, rope_cos, rope_sin, W_qkv, b_qkv, W_out, b_out):
    from concourse.bass_utils import run_bass_kernel_spmd

    with_bias = bool(np.any(np.asarray(b_qkv)) or np.any(np.asarray(b_out)))
    key = ("nc", with_bias)
    if key not in _CACHED:
        _CACHED[key] = build_nc(1, with_bias=with_bias)
        _CACHED["nc"] = _CACHED[key]  # convenience for test harness
    nc = _CACHED[key]
    in_maps = _prep_in_maps(x, rope_cos, rope_sin, W_qkv, b_qkv, W_out, b_out,
                            with_bias=with_bias)
    res = run_bass_kernel_spmd(nc, in_maps, list(range(N_CORES)))
    B = x.shape[0]
    out = np.zeros((B, S, D), dtype=np.float32)
    for c in range(N_CORES):
        b = c // 4
        out[b] += res.results[c]["y"].astype(np.float32)
    return out


# revision 25
# speedup vs baseline: 1.0650x; 1.0650x over previous
"""Trainium2 Bass kernel for nn_Attention (B=2, S=2048, D=1024, H=16).

Sharding: 8 cores = 2 batches x 4 head-groups (4 heads per core).
Each core computes QKV projection for its batch restricted to its 4 heads,
full (non-causal) attention for those heads, and a partial output
projection over its 256 channels. The host sums the 4 partial outputs per
batch.

Design notes (v3, HW ~280us vs 324us baseline; rel err ~5.4e-3):
 - x transposed + bf16-cast on host; wqk/wv/qT/kTe/kTo/v/ering bf16,
   attn/wout f32r, y out bf16 (host upcasts and sums partials in f32).
 - W_q/W_k columns are de-interleaved per head on host ([ev|od] instead
   of interleaved rope pairs) so the DVE rope ops see packed 32-wide
   slices: all-bf16 packed operands hit the DVE 2x_1p mode (~2x faster).
   q/k live in this permuted basis everywhere; logits are invariant.
 - kT is stored as kTe/kTo with zero-padded dead halves so every logits
   matmul contracts over K=128: measured on HW, K=64 matmuls run at half
   throughput (482 vs 251 ns for 512 moving cols).
 - PSUM = two pools of rotating tags (no other banks exist): psl holds
   L0/L1/L2 (stage A qk-pair proj + q transposes; stage B triple-buffered
   per-window logits tiles + divide/out-proj scratch on the idle slot —
   scratch tag (W+2)%3 must avoid kt = 2 mod 3 or it collides with the
   next unit's prologue slot), pso holds O (stage A v proj + k
   transposes; stage B AV accumulator). Triple buffering gives every
   producer->consumer sem hop >= 1 full window of slack.
 - stage A per 512-token group, PE order qk x4 -> Tq(g-1) -> v x4 ->
   Tk(g-1): qk pairs drain (ACT) to bf16 qk_sb, DVE ropes into qk_r;
   transposes are deferred one group so PE never waits on rope; Tk after
   v keeps the O-tag rotation off v's critical path. All PSUM drains on
   ACT (a DVE drain queues behind rope and stalls PE).
 - stage B per (query-chunk, head-pair) unit: per key tile kt one
   1024-wide bf16 exp covers both heads; logits(kt+1) issued before
   AV(kt); AV lags one window (flush_av) so the O-drain -> next-unit-AV
   rotation has slack. ACT runs exp back-to-back: stage B sits at the
   ACT roofline (~1.3us/window on HW).
 - divide: O drained to SBUF at unit end; exact DVE reciprocal (~6.9us
   on HW for 1024 - the approx custom op miscompiles here) runs in DVE
   slack, divide2 (K=1 broadcast matmuls + partition-shifted muls) rides
   at kt==10 of the next unit. Out-proj units ride tag-safe windows
   after attn(qc-1) is fully divided; only qc=3 remains in the tail.
   y is staged per query chunk and DMA'd as 4 big transfers (each HWDGE
   dispatch costs ~1.25us of SEQ time - 32 small DMAs would serialize).
 - DMA: only SP/ACT have HWDGE queues, so: wqk 3 DMAs on ACT, xt(g0) 3
   on SP (finer splits let the first chain start ~2us in), cos|sin
   merged into one [S,64] bf16 tensor on gpsimd SWDGE, wout last.
"""

import numpy as np

S = 2048
D = 1024
HD = 64
H_LOC = 4  # heads per core
N_CORES = 8
TT = 16  # token tiles of 128
G = 4    # token groups of 512
QC = 4   # query chunks of 512
KT = 16  # key tiles of 128

_CACHED = {}


def build_nc(repeats: int = 1, with_bias: bool = False):
    import concourse.bass as bass_mod
    import concourse.mybir as mybir
    from concourse import bacc
    from concourse.tile import TileContext
    f32 = mybir.dt.float32
    f32r = mybir.dt.float32r
    bf16 = mybir.dt.bfloat16
    Exp = mybir.ActivationFunctionType.Exp

    nc = bacc.Bacc("TRN2", target_bir_lowering=False, debug=False,
                   num_devices=N_CORES)

    xt_d = nc.dram_tensor("xt", [D, S], bf16, kind="ExternalInput")
    cs_d = nc.dram_tensor("cs", [S, 64], bf16, kind="ExternalInput")
    wqk_d = nc.dram_tensor("wqk", [D, 512], bf16, kind="ExternalInput")
    wv_d = nc.dram_tensor("wv", [D, 256], bf16, kind="ExternalInput")
    wout_d = nc.dram_tensor("wout", [256, D], f32r, kind="ExternalInput")
    ones_d = nc.dram_tensor("ones", [1, 128], f32r, kind="ExternalInput")
    onescol_d = nc.dram_tensor("onescol", [128, 64], f32r,
                               kind="ExternalInput")
    ident_d = nc.dram_tensor("ident", [128, 128], bf16, kind="ExternalInput")
    if with_bias:
        bqk_d = nc.dram_tensor("bqk", [1, 512], bf16, kind="ExternalInput")
        bv_d = nc.dram_tensor("bv", [1, 256], bf16, kind="ExternalInput")
        bout_d = nc.dram_tensor("bout", [1, D], f32r, kind="ExternalInput")
    y_d = nc.dram_tensor("y", [S, D], bf16, kind="ExternalOutput")

    with TileContext(nc) as tc:
        with (
            tc.tile_pool(name="const", bufs=1) as cpool,
            tc.tile_pool(name="xin", bufs=4) as xpool,
            tc.tile_pool(name="qkr", bufs=4) as qkrpool,
            tc.tile_pool(name="rtmp", bufs=2) as rtpool,
            tc.tile_pool(name="big", bufs=1) as bigpool,
            tc.tile_pool(name="et", bufs=6) as etpool,
            tc.tile_pool(name="yt", bufs=2) as ypool,
            tc.tile_pool(name="ysb", bufs=2) as ysbpool,
            tc.tile_pool(name="tl", bufs=1) as tailpool,
            tc.tile_pool(name="sml", bufs=1) as spool,
            tc.tile_pool(name="psl", bufs=1, space="PSUM") as psl,
            tc.tile_pool(name="pso", bufs=1, space="PSUM") as pso,
        ):
            # ---- constants / weights ----
            wqk_sb = cpool.tile([128, 8, 512], bf16)
            wv_sb = cpool.tile([128, 8, 256], bf16)
            wout_sb = cpool.tile([128, 2, D], f32r)
            cs_sb = cpool.tile([128, TT, 64], bf16)
            ones_sb = cpool.tile([1, 128], f32r)
            onescol_sb = cpool.tile([128, 64], f32r)
            ident = cpool.tile([128, 128], bf16)
            if with_bias:
                bqk_sb = cpool.tile([1, 512], bf16)
                bv_sb = cpool.tile([1, 256], bf16)
                bout_sb = cpool.tile([1, D], f32r)
                ones_bf = cpool.tile([1, 128], bf16)

            # wqk on the ACT queue (2 dispatches, done before the first
            # drain is needed); everything else on gpsimd SWDGE ordered by
            # first use (cs for rope ~4us, wv for v-proj ~7us, ident for
            # transposes ~10us, wout not until stage B).
            wqk_r = wqk_d.ap().rearrange("(i p) c -> p i c", p=128)
            nc.scalar.dma_start(wqk_sb[:, 0:2, :], wqk_r[:, 0:2, :])
            nc.scalar.dma_start(wqk_sb[:, 2:4, :], wqk_r[:, 2:4, :])
            nc.scalar.dma_start(wqk_sb[:, 4:8, :], wqk_r[:, 4:8, :])
            nc.gpsimd.dma_start(cs_sb[:], cs_d.ap().rearrange(
                "(t p) c -> p t c", p=128))
            nc.gpsimd.dma_start(wv_sb[:], wv_d.ap().rearrange(
                "(i p) c -> p i c", p=128))
            nc.gpsimd.dma_start(ident[:], ident_d[:])
            nc.gpsimd.dma_start(onescol_sb[:], onescol_d[:])
            nc.gpsimd.dma_start(ones_sb[:], ones_d[:])
            nc.gpsimd.dma_start(wout_sb[:], wout_d.ap().rearrange(
                "(i p) c -> p i c", p=128))
            if with_bias:
                nc.gpsimd.dma_start(bqk_sb[:], bqk_d[:])
                nc.gpsimd.dma_start(bv_sb[:], bv_d[:])
                nc.gpsimd.dma_start(bout_sb[:], bout_d[:])
                nc.vector.tensor_copy(ones_bf[:], ones_sb[:])

            def bcast8(ap):
                # [p, t, j] -> [p, t, (bcast 8), j]
                return bass_mod.AP(ap.tensor, ap.offset,
                                   [ap.ap[0], ap.ap[1], [0, 8], ap.ap[2]])

            def body(_iv=None):
                qT = bigpool.tile([128, 2, S], bf16, tag="qT")
                # kT split per head parity with zero-padded dead halves so
                # logits matmuls contract over K=128 (K=64 runs at half
                # throughput on HW): kTe rows 0:64 = even head, rows 64:128
                # zero; kTo rows 0:64 zero, rows 64:128 = odd head. The
                # moving operand is then the full 128-row packed qT — the
                # other head's rows hit the zero stationary rows.
                kTe = bigpool.tile([128, 2, S], bf16, tag="kTe")
                kTo = bigpool.tile([128, 2, S], bf16, tag="kTo")
                attn = bigpool.tile([128, 2, S], f32r, tag="attn")
                v_sb = bigpool.tile([128, TT, H_LOC, 65], bf16, tag="v")
                nc.vector.memset(kTe[64:128, :, :], 0.0)
                nc.vector.memset(kTo[0:64, :, :], 0.0)
                nc.vector.tensor_copy(
                    v_sb[:, :, :, 64:65],
                    onescol_sb[:].rearrange("p (t h o) -> p t h o",
                                            h=H_LOC, o=1))

                # PSUM: 8 banks as two pools of rotating tags. psl holds
                # three 2-bank slots L0/L1/L2 (stage A: qk pair proj + q
                # transposes; stage B: triple-buffered logits + ride/bcast
                # scratch on the rotating idle slot). pso holds one 2-bank
                # slot O (stage A: v proj + k transposes; stage B: AV
                # accumulator). HW semaphore latency is ~0.5us per hop, so
                # every producer-consumer pair needs >= 1 window of slack —
                # triple buffering gives logits->exp 2 windows.

                # ================= stage A: projections =================
                def rope_pair(pr, qk_sb, qk_r, g):
                    # rope for token tiles (2*pr, 2*pr+1); all-bf16 packed
                    # operands (host de-interleaved W cols) -> DVE 2x mode
                    tt = g * 4 + 2 * pr
                    ts = slice(2 * pr, 2 * pr + 2)
                    cosp = bcast8(cs_sb[:, tt:tt + 2, 0:32])
                    sinp = bcast8(cs_sb[:, tt:tt + 2, 32:64])
                    srcr = qk_sb[:, ts, :].rearrange(
                        "p t (h half j) -> p half t h j", half=2, j=32)
                    dstr = qk_r[:, ts, :].rearrange(
                        "p t (h half j) -> p half t h j", half=2, j=32)
                    ev, od = srcr[:, 0], srcr[:, 1]
                    t1 = rtpool.tile([128, 2, 8, 32], bf16, tag="t1")
                    t2 = rtpool.tile([128, 2, 8, 32], bf16, tag="t2")
                    nc.vector.tensor_mul(t1[:], od, sinp)
                    nc.vector.tensor_mul(dstr[:, 0], ev, cosp)
                    nc.vector.tensor_sub(dstr[:, 0], dstr[:, 0], t1[:])
                    nc.vector.tensor_mul(t2[:], ev, sinp)
                    nc.vector.tensor_mul(dstr[:, 1], od, cosp)
                    nc.vector.tensor_add(dstr[:, 1], dstr[:, 1], t2[:])

                # Stage A runs as three long dense PE phases (all qk chains,
                # then all transposes, then all v chains) instead of
                # per-group round-robins: the PE p-state only ramps to full
                # speed after ~3us of CONTINUOUS execution, and the chopped
                # per-group stream was measuring ~2x its cycle budget.
                def do_transposes(gm, qk_r, tagq, tagk):
                    gb = slice(gm * 512, (gm + 1) * 512)
                    poolq = pso if tagq == "O" else psl
                    poolk = pso if tagk == "O" else psl
                    Tq = poolq.tile([128, 2, 512], bf16, tag=tagq,
                                    name=f"Tq{gm}",
                                    padded_shape=[128, 2, 1024])
                    Tk = poolk.tile([128, 2, 512], bf16, tag=tagk,
                                    name=f"Tk{gm}",
                                    padded_shape=[128, 2, 1024])
                    for cc in range(4):
                        ps_t = (Tq, Tk)[cc // 2][:, cc % 2, :]
                        for ti in range(4):
                            nc.tensor.transpose(
                                ps_t[:, ti * 128:(ti + 1) * 128],
                                qk_r[:, ti, cc * 128:(cc + 1) * 128],
                                ident[:])
                    nc.scalar.copy(qT[:, 0:2, gb], Tq[:, :, :])
                    # k: rows 0:64 = even head, 64:128 = odd head
                    nc.scalar.copy(kTe[0:64, 0:2, gb], Tk[0:64, :, :])
                    nc.scalar.copy(kTo[64:128, 0:2, gb], Tk[64:128, :, :])

                xts, qkrs = [], []

                # phase 1: all 16 qk chains (one ~33us PE burst); pair
                # drains (ACT) + ropes (DVE) trail behind
                for g in range(G):
                    xt_g = xpool.tile([128, 8, 512], bf16, tag="xt",
                                      name=f"xt{g}")
                    xt_r = xt_d[:, g * 512:(g + 1) * 512].rearrange(
                        "(i p) s -> p i s", p=128)
                    if g == 0:
                        nc.sync.dma_start(xt_g[:, 0:2, :], xt_r[:, 0:2, :])
                        nc.sync.dma_start(xt_g[:, 2:4, :], xt_r[:, 2:4, :])
                        nc.sync.dma_start(xt_g[:, 4:8, :], xt_r[:, 4:8, :])
                    else:
                        nc.sync.dma_start(xt_g[:], xt_r)
                    xts.append(xt_g)

                    qkA = psl.tile([128, 2, 512], f32, tag="L0",
                                   name=f"qkA{g}")
                    qkB = psl.tile([128, 2, 512], f32, tag="L1",
                                   name=f"qkB{g}")
                    qk_sb = tailpool.tile([128, 4, 512], bf16, tag="qks",
                                          name=f"qks{g}")
                    qk_r = qkrpool.tile([128, 4, 512], bf16, tag="qkr",
                                        name=f"qkr{g}")
                    qkrs.append(qk_r)

                    for ti in range(4):
                        ps_qk = (qkA, qkB)[ti // 2][:, ti % 2, :]
                        for fc in range(8):
                            nc.tensor.matmul(
                                ps_qk, xt_g[:, fc, ti * 128:(ti + 1) * 128],
                                wqk_sb[:, fc, :],
                                start=(fc == 0),
                                stop=(not with_bias and fc == 7))
                        if with_bias:
                            nc.tensor.matmul(ps_qk, ones_bf[0:1, 0:128],
                                             bqk_sb[:], start=False, stop=True)
                        if ti % 2 == 1:
                            src = (qkA, qkB)[ti // 2]
                            nc.scalar.copy(
                                qk_sb[:, ti - 1:ti + 1, :], src[:, :, :])
                            rope_pair(ti // 2, qk_sb, qk_r, g)

                # phase 2: all transposes; T tiles rotate over 4 PSUM tags
                # so no transpose ever waits on a drain
                ttags = [("L2", "O"), ("L0", "L1")]
                for g in range(G):
                    do_transposes(g, qkrs[g], *ttags[g % 2])

                # phase 3: all v chains; one ACT drain per group
                for g in range(G):
                    O_a = pso.tile([128, 2, 512], f32, tag="O", name=f"Oa{g}")
                    for ti in range(4):
                        ps_v = O_a[:, ti // 2,
                                   (ti % 2) * 256:(ti % 2) * 256 + 256]
                        for fc in range(8):
                            nc.tensor.matmul(
                                ps_v, xts[g][:, fc, ti * 128:(ti + 1) * 128],
                                wv_sb[:, fc, :],
                                start=(fc == 0),
                                stop=(not with_bias and fc == 7))
                        if with_bias:
                            nc.tensor.matmul(ps_v, ones_bf[0:1, 0:128],
                                             bv_sb[:], start=False, stop=True)
                    nc.scalar.copy(
                        v_sb[:, g * 4:(g + 1) * 4, :, 0:64],
                        O_a[:, :, :].rearrange(
                            "p s (t h d) -> p (s t) h d", t=2, h=H_LOC, d=64))

                # ============ stage B: attention (+ C interleaved) ============
                units = [(qc, hp) for qc in range(QC) for hp in range(2)]
                ltiles = {}          # global window index -> logits tile
                scratch_n = [0]      # rotating tag counter for tail psum

                def logits(qc, hp, kt, W):
                    t = psl.tile([128, 2, 512], f32, tag=f"L{W % 3}",
                                 name=f"Lg{W}")
                    ltiles[W] = t
                    for (j, ktp) in ((0, kTe), (1, kTo)):
                        nc.tensor.matmul(
                            t[:, j, :],
                            ktp[:, hp, kt * 128:(kt + 1) * 128],
                            qT[:, hp, qc * 512:(qc + 1) * 512],
                            start=True, stop=True)

                def scratch(W, name):
                    # the idle L slot at window W (exp(W-1) just released it,
                    # logits(W+2) won't claim it until next window)
                    if W is None:
                        scratch_n[0] += 1
                        return psl.tile([128, 2, 512], f32,
                                        tag=f"L{scratch_n[0] % 3}", name=name)
                    return psl.tile([128, 2, 512], f32, tag=f"L{(W + 2) % 3}",
                                    name=name)

                ysb_state = {}   # qc2 -> [tile, n_done]

                def c_unit(qc2, u, W, drain_eng):
                    # full out-proj unit (both halves + drain into the
                    # chunk-local y staging tile; chunk DMA after the 8th)
                    if qc2 not in ysb_state:
                        ysb_state[qc2] = [ysbpool.tile([128, 4, 1024], bf16,
                                                       tag="ysb",
                                                       name=f"ysb{qc2}"), 0]
                    y_sb = ysb_state[qc2][0]
                    ti, ec = divmod(u, 2)
                    tt = qc2 * 4 + ti
                    y_ps = scratch(W, f"ycu{qc2}_{u}")[:, 0, :]
                    for half in range(2):
                        nc.tensor.matmul(
                            y_ps, attn[:, half, tt * 128:(tt + 1) * 128],
                            wout_sb[:, half, ec * 512:(ec + 1) * 512],
                            start=(half == 0),
                            stop=(half == 1 and not with_bias))
                    if with_bias:
                        nc.tensor.matmul(y_ps, ones_sb[0:1, 0:128],
                                         bout_sb[0:1, ec * 512:(ec + 1) * 512],
                                         start=False, stop=True)
                    dst = y_sb[:, ti, ec * 512:(ec + 1) * 512]
                    if drain_eng == "act":
                        nc.scalar.copy(dst, y_ps)
                    else:
                        nc.vector.tensor_copy(dst, y_ps)
                    ysb_state[qc2][1] += 1
                    if ysb_state[qc2][1] == 8:
                        nc.sync.dma_start(
                            y_d[qc2 * 512:(qc2 + 1) * 512, :].rearrange(
                                "(t p) c -> p t c", p=128),
                            y_sb[:])

                def divide1(ui, qc, hp, O):
                    # Drain O to SBUF immediately (frees O for the next
                    # unit's AV accumulation) and take the reciprocal of
                    # the sums row — all DVE, off the PE critical path.
                    o_sb = spool.tile([65, 2, 512], f32r, tag="osb",
                                      name=f"osb{ui}")
                    nc.vector.tensor_copy(o_sb[:, :, :], O[0:65, :, :])
                    rec = spool.tile([1, 1024], f32r, tag="rec",
                                     name=f"rec{ui}")
                    with nc.allow_low_precision(
                            reason="f32r reciprocal feeds f32r multiply"):
                        # exact DVE reciprocal is ~6 cyc/elem on HW (~6.9us
                        # for 1024) but attn is only needed at the tail now,
                        # so divide2 rides late (kt==8) and the DVE queue has
                        # ~10us of slack per unit to absorb it.
                        nc.vector.reciprocal(rec[0:1, :],
                                             o_sb[64:65, :, :].rearrange(
                                                 "p a b -> p (a b)"))
                    return (ui, qc, hp, o_sb, rec)

                def divide2(ui, qc, hp, o_sb, rec, W):
                    # K=1 broadcast matmuls into the two banks of one
                    # scratch slot, then partition-shifted DVE multiplies.
                    bcb = scratch(W, f"bcb{ui}")
                    nc.tensor.matmul(bcb[0:64, 0, :], ones_sb[0:1, 0:64],
                                     rec[0:1, 0:512], start=True, stop=True)
                    nc.tensor.matmul(bcb[0:64, 1, :], ones_sb[0:1, 0:64],
                                     rec[0:1, 512:1024], start=True, stop=True)
                    qs = slice(qc * 512, (qc + 1) * 512)
                    nc.vector.tensor_mul(attn[0:64, hp, qs],
                                         o_sb[0:64, 0, :], bcb[0:64, 0, :])
                    nc.vector.tensor_mul(attn[64:128, hp, qs],
                                         o_sb[0:64, 1, :], bcb[0:64, 1, :])

                logits(0, 0, 0, 0)  # prologue for the first unit
                pdiv = [None]
                # AV matmuls are issued one window late (AV(W-1) inside
                # window W) so the O-drain -> next-unit-AV tag rotation has
                # a full window of slack instead of stalling PE ~1.7us at
                # every unit boundary.
                prev_av = [None]

                def flush_av():
                    O2, ui2, qc2, hp2, kt2, er2 = prev_av[0]
                    nc.tensor.matmul(
                        O2[0:65, 0, :], v_sb[:, kt2, 2 * hp2, :],
                        er2[:, 0, :], start=(kt2 == 0), stop=(kt2 == KT - 1))
                    nc.tensor.matmul(
                        O2[0:65, 1, :], v_sb[:, kt2, 2 * hp2 + 1, :],
                        er2[:, 1, :], start=(kt2 == 0), stop=(kt2 == KT - 1))
                    if kt2 == KT - 1:
                        pdiv[0] = divide1(ui2, qc2, hp2, O2)

                for ui, (qc, hp) in enumerate(units):
                    O = pso.tile([128, 2, 512], f32, tag="O", name=f"O{ui}")
                    if hp == 0 and qc > 0:
                        rides = {12: (qc - 1, 0), 13: (qc - 1, 1)}
                    elif hp == 1 and qc > 0:
                        rides = dict(zip((3, 4, 6, 7, 9, 12),
                                         ((qc - 1, u) for u in range(2, 8))))
                    else:
                        rides = {}
                    for kt in range(KT):
                        W = ui * KT + kt
                        ering = etpool.tile([128, 2, 512], bf16, tag="er")
                        src = ltiles.pop(W)
                        nc.scalar.activation(ering[:], src[:, :, :],
                                             Exp, scale=0.125)
                        if kt < KT - 1:
                            logits(qc, hp, kt + 1, W + 1)
                        elif ui + 1 < len(units):
                            qc2, hp2 = units[ui + 1]
                            with tc.high_priority():
                                logits(qc2, hp2, 0, W + 1)
                        # the previous unit's softmax divide rides at kt==10:
                        # its scratch tag must avoid kt = 2 mod 3 (those
                        # collide with the next unit's prologue logits slot)
                        if kt == 10 and pdiv[0] is not None:
                            divide2(*pdiv[0], W)
                            pdiv[0] = None
                        if kt in rides:
                            qc2, u = rides[kt]
                            c_unit(qc2, u, W, "act" if u % 2 else "dve")
                        if prev_av[0] is not None:
                            flush_av()
                        prev_av[0] = (O, ui, qc, hp, kt, ering)

                flush_av()  # the last window's AV + divide1

                # ---- tail: the last chunk's divide + out-proj ----
                divide2(*pdiv[0], None)
                pdiv[0] = None
                for u in range(8):
                    c_unit(QC - 1, u, None, "act" if u % 2 else "dve")

            if repeats == 1:
                body()
            else:
                with tc.For_i(0, repeats, 1) as _i:
                    body(_i)

    nc.compile()
    return nc


# de-interleave rope pairs within each head's 64 channels: [ev(32)|od(32)]
_DEINT = np.concatenate([np.arange(0, 64, 2), np.arange(1, 64, 2)])


def _prep_in_maps(x, rope_cos, rope_sin, W_qkv, b_qkv, W_out, b_out,
                  with_bias=False):
    import ml_dtypes
    f32 = np.float32
    bfl = ml_dtypes.bfloat16
    W3 = np.asarray(W_qkv, dtype=f32).reshape(D, 16, 3, HD)  # [f, head, qkv, d]
    b3 = np.asarray(b_qkv, dtype=f32).reshape(16, 3, HD)
    cs = np.concatenate([np.asarray(rope_cos, dtype=f32),
                         np.asarray(rope_sin, dtype=f32)], axis=1)
    cs = np.ascontiguousarray(cs).astype(bfl)
    ones = np.ones((1, 128), dtype=f32)
    onescol = np.ones((128, 64), dtype=f32)
    W_out = np.asarray(W_out, dtype=f32)
    b_out = np.asarray(b_out, dtype=f32)
    x = np.asarray(x, dtype=f32)

    in_maps = []
    for c in range(N_CORES):
        b, hg = divmod(c, 4)
        hs = slice(hg * H_LOC, (hg + 1) * H_LOC)
        wq = W3[:, hs, 0, :][:, :, _DEINT].reshape(D, 256)
        wk = W3[:, hs, 1, :][:, :, _DEINT].reshape(D, 256)
        wv = W3[:, hs, 2, :].reshape(D, 256)
        m = {
            "xt": np.ascontiguousarray(x[b].T).astype(bfl),
            "cs": cs,
            "wqk": np.ascontiguousarray(
                np.concatenate([wq, wk], axis=1)).astype(bfl),
            "wv": np.ascontiguousarray(wv).astype(bfl),
            "wout": np.ascontiguousarray(W_out[hg * 256:(hg + 1) * 256, :]),
            "ones": ones, "onescol": onescol,
            "ident": np.eye(128, dtype=f32).astype(bfl),
        }
        if with_bias:
            bq = b3[hs, 0, :][:, _DEINT].reshape(1, 256)
            bk = b3[hs, 1, :][:, _DEINT].reshape(1, 256)
            m["bqk"] = np.ascontiguousarray(
                np.concatenate([bq, bk], axis=1)).astype(bfl)
            m["bv"] = np.ascontiguousarray(
                b3[hs, 2, :].reshape(1, 256)).astype(bfl)
            m["bout"] = (np.ascontiguousarray(b_out.reshape(1, D)) if hg == 0
                         else np.zeros((1, D), dtype=f32))
        in_maps.append(m)
    return in_maps


def kernel(x, rope_cos, rope_sin, W_qkv, b_qkv, W_out, b_out):
    from concourse.bass_utils import run_bass_kernel_spmd

    with_bias = bool(np.any(np.asarray(b_qkv)) or np.any(np.asarray(b_out)))
    key = ("nc", with_bias)
    if key not in _CACHED:
        _CACHED[key] = build_nc(1, with_bias=with_bias)
        _CACHED["nc"] = _CACHED[key]  # convenience for test harness
    nc = _CACHED[key]
    in_maps = _prep_in_maps(x, rope_cos, rope_sin, W_qkv, b_qkv, W_out, b_out,
                            with_bias=with_bias)
    res = run_bass_kernel_spmd(nc, in_maps, list(range(N_CORES)))
    B = x.shape[0]
    out = np.zeros((B, S, D), dtype=np.float32)
    for c in range(N_CORES):
        b = c // 4
        out[b] += res.results[c]["y"].astype(np.float32)
    return out
